# revision 14
# baseline (speedup 1.0000x reference)
"""Trainium2 Bass kernel for nn_CRNet (gnn_message_passing).

Math (reference):
  vc   = relu(vf @ W_v1 + b_v1) @ W_v2 + b_v2                 # [B,D]
  clu  = relu(cc @ W_v1 + b_v1) @ W_v2 + b_v2                 # [K,D]
  sp   = relu(cp @ W_s1 + b_s1) @ W_s2 + b_s2                 # [C,D]
  out1[p,:] = sum_{k,e} relu((sp[p]-clu[k]) @ W_exp[e] + b_exp[e])   # [C,D]
  out2[b,c] = relu(vc[b]@Wa + out1[c]@Wb + b_r1) @ w2 + b_r2         # [B,C]

Factorization used on-device:
  A''[e] = sp @ W_exp[e] + b_exp[e]   (per-expert linear map, small matmuls)
  Dm[e]  = -(clu @ W_exp[e])
  out1[p,d'] = sum_{k,e} relu(A''[e][p,d'] + Dm[e][k,d'])     <- fused bias+relu
  out2[b,c]  = sum_d' w2[d'] relu(VA[b,d'] + S''[c,d']) + br2 <- fused bias+relu
with the fused ops laid out [d' (partitions), batch/class (free)] so the
bias term is a per-partition scalar, relu+bias fuse into one DVE
tensor_scalar (fp16, 4x mode) or one ACT activation, and the reductions
run on the PE (identity-matmul PSUM accumulation / M=1 matmuls with w2
as stationary weights, 4-way column-tiled).

Sharding over 8 cores (SPMD, per-core data via in_maps):
  - block1: (expert-half x cluster-quarter) grid -> 3 experts x 25 clusters/core
  - out1 partials: transpose -> HBM -> ReduceScatter -> 32 classes/core
  - block2: c-sharded (32 classes/core, free dim = all 1024 b)
  - visual pipeline: b-sharded (128 b/core) + AllGather of fp16 VA'_T
"""

import numpy as np

B, C, K, VD, SD, D, E = 1024, 256, 100, 64, 200, 256, 6
NCORES = 8
BSH = B // NCORES      # 128 b per core (stage0a shard)
CSH = C // NCORES      # 32 classes per core (block2 shard)
EH = 3                 # experts per core (expert half)
KQ = 25                # clusters per core (cluster quarter)
DT = 2                 # number of 128-partition tiles covering D=256

# engine split: fraction of fused units on DVE vs ACT (cost-balanced)
BLK1_ACT_EVERY = 4     # every 4th block1 unit goes to ACT
BLK2_ACT_EVERY = 4     # every 4th block2 unit goes to ACT


def _build_program():
    import concourse.bass as bass
    import concourse.bacc as bacc
    import concourse.mybir as mybir
    from concourse import tile

    f32 = mybir.dt.float32
    f16 = mybir.dt.float16
    AF = mybir.ActivationFunctionType
    OP = mybir.AluOpType

    nc = bacc.Bacc(
        "TRN2",
        target_bir_lowering=False,
        debug=False,
        enable_asserts=False,
        num_devices=NCORES,
    )

    # ---------------- DRAM I/O ----------------
    def inp(name, shape, dt=f32):
        return nc.dram_tensor(name, shape, dt, kind="ExternalInput").ap()

    vfT = inp("vfT", [VD, BSH])            # vf[b-shard].T
    cpT = inp("cpT", [SD, C])              # class_prototypes.T (replicated)
    cluT = inp("cluT", [VD, KQ])           # cluster quarter, transposed
    Wv1 = inp("Wv1", [VD, D])
    Wv2 = inp("Wv2", [D, D])
    Ws1 = inp("Ws1", [SD, D])
    Ws2 = inp("Ws2", [D, D])
    Wa = inp("Wa", [D, D])
    Wb = inp("Wb", [D, D])
    Wexp = inp("Wexp", [EH, D, D])         # this core's 3 experts
    bv1 = inp("bv1", [128, DT])            # biases reshaped (2,128).T
    bv2 = inp("bv2", [128, DT])
    bs1 = inp("bs1", [128, DT])
    bs2 = inp("bs2", [128, DT])
    br1 = inp("br1", [128, DT])
    bexp = inp("bexp", [128, EH * DT])     # col = e*2 + t
    w2h = inp("w2h", [128, DT * 32], f16)  # W_r2 tiles replicated 32x, fp16
    br2c = inp("br2c", [128, 1])           # b_r2 broadcast
    idh = inp("idh", [128, 128], f16)      # identity fp16
    idf = inp("idf", [128, 128])           # identity f32

    out2 = nc.dram_tensor("out2", [CSH, B], f32, kind="ExternalOutput").ap()

    # internal DRAM for collectives
    va_chunk = nc.dram_tensor("va_chunk", [D, BSH], f16).ap()
    va_all = nc.dram_tensor("va_all", [NCORES * D, BSH], f16,
                            addr_space="Shared").ap()
    out1_full = nc.dram_tensor("out1_full", [C, D], f32).ap()
    out1_mine = nc.dram_tensor("out1_mine", [CSH, D], f32).ap()

    groups = [list(range(NCORES))]

    with tile.TileContext(nc) as tc:
        with (
            tc.tile_pool(name="const", bufs=1) as cpool,
            tc.tile_pool(name="work", bufs=3) as wpool,
            tc.tile_pool(name="h1", bufs=6) as h1pool,
            tc.tile_pool(name="h2", bufs=12) as h2pool,
            tc.tile_pool(name="ps", bufs=2, space="PSUM") as pspool,
            tc.tile_pool(name="acc", bufs=1, space="PSUM") as accpool,
        ):
            # ---------------- load constants to SBUF ----------------
            def load(ap_dram, shape, dt=f32, tag=None, part=None):
                t = cpool.tile(shape, dt, tag=tag)
                dst = t[:part] if part is not None else t[:]
                nc.sync.dma_start(out=dst, in_=ap_dram)
                return t

            # weight [256, D] -> sbuf [128, (t, m)]: col t*D+m = W[t*128+p, m]
            def load_w2t(ap_dram, tag):
                t = cpool.tile([128, DT * D], f32, tag=tag)
                for kt in range(DT):
                    nc.sync.dma_start(out=t[:, kt * D:(kt + 1) * D],
                                      in_=ap_dram[kt * 128:(kt + 1) * 128, :])
                return t

            wv1_sb = load(Wv1, [VD, D], tag="wv1", part=VD)
            wv2_sb = load_w2t(Wv2, "wv2")
            ws2_sb = load_w2t(Ws2, "ws2")
            wa_sb = load_w2t(Wa, "wa")
            wb_sb = load_w2t(Wb, "wb")
            # W_s1 [200, 256]: rows 0:128 and 128:200
            ws1a_sb = load(Ws1[0:128, :], [128, D], tag="ws1a")
            ws1b_sb = load(Ws1[128:SD, :], [128, D], tag="ws1b", part=SD - 128)
            wexp_sb = [load_w2t(Wexp[e], f"wexp{e}") for e in range(EH)]

            vfT_sb = load(vfT, [VD, BSH], tag="vfT", part=VD)
            cpT0_sb = load(cpT[0:128, :], [128, C], tag="cpT0")
            cpT1_sb = load(cpT[128:SD, :], [128, C], tag="cpT1", part=SD - 128)
            cluT_sb = load(cluT, [VD, KQ], tag="cluT", part=VD)

            bv1_sb = load(bv1, [128, DT], tag="bv1")
            bv2_sb = load(bv2, [128, DT], tag="bv2")
            bs1_sb = load(bs1, [128, DT], tag="bs1")
            bs2_sb = load(bs2, [128, DT], tag="bs2")
            br1_sb = load(br1, [128, DT], tag="br1")
            bexp_sb = load(bexp, [128, EH * DT], tag="bexp")
            w2h_sb = load(w2h, [128, DT * 32], f16, tag="w2h")
            br2_sb = load(br2c, [128, 1], tag="br2c")
            idh_sb = load(idh, [128, 128], f16, tag="idh")
            idf_sb = load(idf, [128, 128], tag="idf")

            def wslice(wsb, kt, mt):  # [128,128] lhsT block (K-tile kt, M-tile mt)
                return wsb[:, kt * D + mt * 128: kt * D + mt * 128 + 128]

            # 2-layer mapper:  inT [K<=128 (vd), N] -> out_T [d(2 tiles), N]
            # layer1: relu(W1.T @ inT + b1); layer2: W2.T @ r + b2 (via epilogue)
            def mapper_visual(inT_ap, n, tag):
                r1 = wpool.tile([128, DT * n], f32, tag=f"{tag}_r1")
                for mt in range(DT):
                    ps = pspool.tile([128, 512], f32, tag="ps_map")
                    nc.tensor.matmul(ps[:, :n], wv1_sb[:VD, mt * 128:(mt + 1) * 128],
                                     inT_ap, start=True, stop=True)
                    nc.scalar.activation(r1[:, mt * n:(mt + 1) * n], ps[:, :n],
                                         AF.Relu, bias=bv1_sb[:, mt:mt + 1])
                outs = []
                for mt in range(DT):
                    ps = pspool.tile([128, 512], f32, tag="ps_map")
                    for kt in range(DT):
                        nc.tensor.matmul(ps[:, :n], wslice(wv2_sb, kt, mt),
                                         r1[:, kt * n:(kt + 1) * n],
                                         start=(kt == 0), stop=(kt == DT - 1))
                    o = wpool.tile([128, n], f32, tag=f"{tag}_o{mt}")
                    nc.scalar.activation(o[:], ps[:, :n], AF.Identity,
                                         bias=bv2_sb[:, mt:mt + 1])
                    outs.append(o)
                return outs  # list of DT tiles [128, n] = mapped_T

            # ---- stage0a: visual features (b-shard) -> VA'_T -> AllGather ----
            vc_T = mapper_visual(vfT_sb[:VD, :], BSH, "vc")
            for mt in range(DT):
                ps = pspool.tile([128, 512], f32, tag="ps_map")
                for kt in range(DT):
                    nc.tensor.matmul(ps[:, :BSH], wslice(wa_sb, kt, mt),
                                     vc_T[kt][:], start=(kt == 0), stop=(kt == DT - 1))
                va16 = wpool.tile([128, BSH], f16, tag=f"va16_{mt}")
                nc.scalar.activation(va16[:], ps[:, :BSH], AF.Identity,
                                     bias=br1_sb[:, mt:mt + 1])
                nc.sync.dma_start(out=va_chunk[mt * 128:(mt + 1) * 128, :],
                                  in_=va16[:])
            nc.gpsimd.collective_compute(
                "AllGather", OP.bypass, replica_groups=groups,
                ins=[va_chunk[:]], outs=[va_all[:]])
            # load back VA'_T [d' tile, all B]  (b = chunk*128 + j)
            vaT = []
            for t in range(DT):
                v = cpool.tile([128, B], f16, tag=f"vaT{t}")
                nc.sync.dma_start(
                    out=v[:].rearrange("p (c j) -> p c j", c=NCORES),
                    in_=va_all.rearrange("(c u p) j -> u p c j", u=DT, p=128)[t])
                vaT.append(v)

            # ---- stage0e: cluster quarter -> clu_T -> Dm[e] ----
            clu_T = mapper_visual(cluT_sb[:VD, :], KQ, "clu")
            Dm = []  # Dm[e][t]: [128, KQ] f32 = -(clu @ W_exp[e]).T tile
            for e in range(EH):
                row = []
                for mt in range(DT):
                    ps = pspool.tile([128, 512], f32, tag="ps_map")
                    for kt in range(DT):
                        nc.tensor.matmul(ps[:, :KQ], wslice(wexp_sb[e], kt, mt),
                                         clu_T[kt][:],
                                         start=(kt == 0), stop=(kt == DT - 1))
                    d_t = cpool.tile([128, KQ], f32, tag=f"Dm{e}_{mt}")
                    nc.scalar.activation(d_t[:], ps[:, :KQ], AF.Identity,
                                         bias=0.0, scale=-1.0)
                    row.append(d_t)
                Dm.append(row)

            # ---- stage0c: semantic prototypes -> sem_pre_T ----
            rs1 = wpool.tile([128, DT * C], f32, tag="rs1")
            for mt in range(DT):
                ps = pspool.tile([128, 512], f32, tag="ps_map")
                nc.tensor.matmul(ps[:, :C], ws1a_sb[:, mt * 128:(mt + 1) * 128],
                                 cpT0_sb[:], start=True, stop=False)
                nc.tensor.matmul(ps[:, :C], ws1b_sb[:SD - 128, mt * 128:(mt + 1) * 128],
                                 cpT1_sb[:SD - 128, :], start=False, stop=True)
                nc.scalar.activation(rs1[:, mt * C:(mt + 1) * C], ps[:, :C],
                                     AF.Relu, bias=bs1_sb[:, mt:mt + 1])
            semp = []
            for mt in range(DT):
                ps = pspool.tile([128, 512], f32, tag="ps_map")
                for kt in range(DT):
                    nc.tensor.matmul(ps[:, :C], wslice(ws2_sb, kt, mt),
                                     rs1[:, kt * C:(kt + 1) * C],
                                     start=(kt == 0), stop=(kt == DT - 1))
                s = wpool.tile([128, C], f32, tag=f"semp{mt}")
                nc.scalar.activation(s[:], ps[:, :C], AF.Identity,
                                     bias=bs2_sb[:, mt:mt + 1])
                semp.append(s)

            # ---- stage0d: A''[e] = (sem_pre @ W_exp[e] + b_exp[e]).T  (fp16) ----
            A16 = []
            for e in range(EH):
                row = []
                for mt in range(DT):
                    ps = pspool.tile([128, 512], f32, tag="ps_map")
                    for kt in range(DT):
                        nc.tensor.matmul(ps[:, :C], wslice(wexp_sb[e], kt, mt),
                                         semp[kt][:],
                                         start=(kt == 0), stop=(kt == DT - 1))
                    a = cpool.tile([128, C], f16, tag=f"A16_{e}_{mt}")
                    nc.scalar.activation(a[:], ps[:, :C], AF.Identity,
                                         bias=bexp_sb[:, e * DT + mt:e * DT + mt + 1])
                    row.append(a)
                A16.append(row)

            # ---- block1: out1_T[t] = sum_{e,k} relu(A16[e][t] + Dm[e][t][:,k]) ----
            pacc = [accpool.tile([128, C], f32, tag=f"pacc{t}", name=f"pacc{t}")
                    for t in range(DT)]
            ucount = 0
            for t in range(DT):
                n_units = EH * KQ
                u = 0
                for e in range(EH):
                    for k in range(KQ):
                        h = h1pool.tile([128, C], f16, tag="h1")
                        if ucount % BLK1_ACT_EVERY == BLK1_ACT_EVERY - 1:
                            nc.scalar.activation(h[:], A16[e][t][:], AF.Relu,
                                                 bias=Dm[e][t][:, k:k + 1])
                        else:
                            nc.vector.tensor_scalar(
                                h[:], A16[e][t][:], Dm[e][t][:, k:k + 1], 0.0,
                                OP.add, OP.max)
                        nc.tensor.matmul(pacc[t][:], idh_sb[:], h[:],
                                         start=(u == 0), stop=(u == n_units - 1))
                        u += 1
                        ucount += 1

            # drain accumulators, transpose to [p, d], send to HBM, ReduceScatter
            o1 = []
            for t in range(DT):
                o = wpool.tile([128, C], f32, tag=f"o1_{t}")
                nc.vector.tensor_copy(o[:], pacc[t][:])
                o1.append(o)
            for pt in range(DT):
                optile = wpool.tile([128, D], f32, tag=f"o1pd_{pt}")
                for t in range(DT):
                    ps = pspool.tile([128, 512], f32, tag="ps_map")
                    nc.tensor.transpose(ps[:, :128],
                                        o1[t][:, pt * 128:(pt + 1) * 128],
                                        idf_sb[:])
                    nc.vector.tensor_copy(optile[:, t * 128:(t + 1) * 128],
                                          ps[:, :128])
                nc.sync.dma_start(out=out1_full[pt * 128:(pt + 1) * 128, :],
                                  in_=optile[:])
            nc.gpsimd.collective_compute(
                "ReduceScatter", OP.add, replica_groups=groups,
                ins=[out1_full[:]], outs=[out1_mine[:]])

            # load own 32 classes, transpose, S''_T = Wb.T @ out1_mine_T
            om = wpool.tile([128, D], f32, tag="om")
            nc.sync.dma_start(out=om[:CSH, :], in_=out1_mine[:])
            omT = []
            for t in range(DT):
                ps = pspool.tile([128, 512], f32, tag="ps_map")
                nc.tensor.transpose(ps[:, :CSH], om[:CSH, t * 128:(t + 1) * 128],
                                    idf_sb[:CSH, :CSH])
                oT = wpool.tile([128, CSH], f32, tag=f"omT{t}")
                nc.vector.tensor_copy(oT[:], ps[:, :CSH])
                omT.append(oT)
            S2 = []
            for mt in range(DT):
                ps = pspool.tile([128, 512], f32, tag="ps_map")
                for kt in range(DT):
                    nc.tensor.matmul(ps[:, :CSH], wslice(wb_sb, kt, mt),
                                     omT[kt][:], start=(kt == 0), stop=(kt == DT - 1))
                s2 = wpool.tile([128, CSH], f32, tag=f"S2_{mt}")
                nc.vector.tensor_copy(s2[:], ps[:, :CSH])
                S2.append(s2)

            # ---- block2: out2[c, b] = sum w2 relu(vaT + S2[:,c]) + br2 ----
            # c = 4g + j -> psum strip partition 32j of group g's psum tile
            with tc.tile_pool(name="psb2", bufs=2, space="PSUM") as psb2:
                ucount = 0
                for g in range(CSH // 4):
                    pg = psb2.tile([128, B], f32, tag="pg", name=f"pg{g}")
                    hh = {}
                    for j in range(4):
                        c = 4 * g + j
                        for t in range(DT):
                            h = h2pool.tile([128, B], f16, tag="h2",
                                            name=f"h2_{c}_{t}")
                            if ucount % BLK2_ACT_EVERY == BLK2_ACT_EVERY - 1:
                                nc.scalar.activation(h[:], vaT[t][:], AF.Relu,
                                                     bias=S2[t][:, c:c + 1])
                            else:
                                nc.vector.tensor_scalar(
                                    h[:], vaT[t][:], S2[t][:, c:c + 1], 0.0,
                                    OP.add, OP.max)
                            hh[(j, t)] = h
                        ucount += 1
                    for j in range(4):
                        for ch in range(2):
                            for t in range(DT):
                                nc.tensor.matmul(
                                    pg[32 * j:32 * j + 32, ch * 512:(ch + 1) * 512],
                                    w2h_sb[:, t * 32:(t + 1) * 32],
                                    hh[(j, t)][:, ch * 512:(ch + 1) * 512],
                                    start=(t == 0), stop=(t == DT - 1),
                                    tile_position=(0, 32 * j),
                                    skip_group_check=True)
                    # drain + bias (rows replicated 32x within each strip)
                    osb = cpool.tile([128, B], f32, tag=f"osb{g}", name=f"osb{g}")
                    if g % 2 == 0:
                        nc.vector.tensor_scalar_add(osb[:], pg[:], br2_sb[:])
                    else:
                        nc.scalar.activation(osb[:], pg[:], AF.Identity,
                                             bias=br2_sb[:])
                    nc.sync.dma_start(
                        out=out2[4 * g:4 * g + 4, :],
                        in_=osb.rearrange("(s r) n -> s r n", r=32)[:, 0, :])

    nc.compile()
    return nc


def _prepare_in_maps(inputs):
    """Host-side sharding/layout: slices, transposes, reshapes only."""
    f = lambda x: np.ascontiguousarray(x, dtype=np.float32)
    vf = f(inputs["visual_features"])
    cp = f(inputs["class_prototypes"])
    cc = f(inputs["cluster_centers"])
    W_r1 = f(inputs["W_r1"])
    b2 = lambda b: np.ascontiguousarray(f(b).reshape(DT, 128).T)
    common = dict(
        cpT=np.ascontiguousarray(cp.T),
        Wv1=f(inputs["W_v1"]), Wv2=f(inputs["W_v2"]),
        Ws1=f(inputs["W_s1"]), Ws2=f(inputs["W_s2"]),
        Wa=np.ascontiguousarray(W_r1[:D]), Wb=np.ascontiguousarray(W_r1[D:]),
        bv1=b2(inputs["b_v1"]), bv2=b2(inputs["b_v2"]),
        bs1=b2(inputs["b_s1"]), bs2=b2(inputs["b_s2"]),
        br1=b2(inputs["b_r1"]),
        w2h=np.ascontiguousarray(np.repeat(
            f(inputs["W_r2"]).reshape(DT, 128).T.astype(np.float16),
            32, axis=1)),
        br2c=np.full((128, 1), float(np.asarray(inputs["b_r2"]).reshape(-1)[0]),
                     np.float32),
        idh=np.eye(128, dtype=np.float16),
        idf=np.eye(128, dtype=np.float32),
    )
    W_exp = f(inputs["W_exp"])
    b_exp = f(inputs["b_exp"])
    in_maps = []
    for i in range(NCORES):
        h, q = i // 4, i % 4
        bexp_cols = np.ascontiguousarray(
            b_exp[EH * h:EH * h + EH].reshape(EH * DT, 128).T)
        m = dict(common)
        m.update(
            vfT=np.ascontiguousarray(vf[BSH * i:BSH * (i + 1)].T),
            cluT=np.ascontiguousarray(cc[KQ * q:KQ * (q + 1)].T),
            Wexp=np.ascontiguousarray(W_exp[EH * h:EH * h + EH]),
            bexp=bexp_cols,
        )
        in_maps.append(m)
    return in_maps


def _assemble(results):
    cols = np.concatenate([results[i]["out2"] for i in range(NCORES)], axis=0)
    return np.ascontiguousarray(cols.T, dtype=np.float32)  # [B, C]


_CACHED = {}


def kernel(**inputs) -> np.ndarray:
    from concourse.bass_utils import run_bass_kernel_spmd
    if "nc" not in _CACHED:
        _CACHED["nc"] = _build_program()
    nc = _CACHED["nc"]
    in_maps = _prepare_in_maps(inputs)
    res = run_bass_kernel_spmd(nc, in_maps, core_ids=list(range(NCORES)))
    return _assemble(res.results)
